# revision 18
# baseline (speedup 1.0000x reference)
"""Trainium2 Bass kernel for CrossAttention (b=2, n=m=2048, dim=1024, 16 heads x 64).

Sharding: 8 cores = (2 batches) x (4 head-groups of 4 heads). Each core computes
q/k/v projections for its 4 heads, rotary, attention, and a partial output
projection y_part = O_heads @ Wo[head_rows]; host sums the 4 partials per batch
and adds bo.

Device-side layout (all transposed, features on partitions — no on-device
transposes anywhere):
  qT/kT [d(=64*2 per pair), n]  <- Wq^T @ x^T     (lhsT=Wq slice, rhs=x^T)
  S^T_j [128 ctx-tok, 2x512 q] <- k_j as lhsT, qT as rhs (two heads row-tiled)
  U = exp(S^T * scale)          (ScalarE, PSUM->SBUF bf16, fd=1024)
  po accumulate [65, 512]       <- lhsT=[v_j | 1], rhs=U  (denominator row free)
  O^T = po[0:64] * (1/po[64])   (DVE recip + GPSIMD bcast + DVE mult)
  y = (O^T).T @ Wo_rows         (lhsT=O^T tile, rhs=Wo rows), PSUM->HBM direct

The kernel is ScalarE(exp)-bound (128 exps at fd=1024 ~ 137us/core). The
schedule keeps ACT saturated: a global j-step stream over 8 attention passes,
with S+exp emitted at their step, AV emitted with a per-pass lag (so v tiles
woven as filler are ready before the PE stream reaches the AV — per-engine
streams execute in emission order), and all projection/v/y work drained as
budgeted filler inside the stream. DMAs are chunked by token-half and ordered
so the first pass starts after ~4MB. PSUM: sps 2x4KB | po 3x2KB | pjy 1x2KB;
the first-allocated po slot of each pass is staged to SBUF right after AV_15
so the next-next pass's AV never waits on the (slow) normalize chain.
Rotary pair-swap is a 32-lane stream_shuffle on VectorE; the +/- sign pattern
is folded into the precomputed sin table (host side); the rotary add runs on
GPSIMD. Masks are all-True -> softmax unmasked.
"""

import collections
import functools

import numpy as np
import ml_dtypes

import jax
from jax.experimental.shard_map import shard_map
from jax.sharding import Mesh, PartitionSpec

import concourse.bass as bass
import concourse.tile as tile
from concourse import bacc, bass2jax, mybir
from concourse.bass2jax import _bass_exec_p, install_neuronx_cc_hook

BF16 = ml_dtypes.bfloat16

B, N, DIM = 2, 2048, 1024
HEADS, DH = 16, 64
G = 4               # heads per core
N_CORES = 8
SCALE = DH ** -0.5
KSUB = DIM // 128   # 8
NT = N // 128       # 16 ctx token tiles
SWAP_MASK = [i ^ 1 for i in range(32)]

_cached = {}


def _build_program(reps=1):
    """Build the SPMD Bass/Tile program (identical on all 8 cores)."""
    fp32 = mybir.dt.float32
    bf16 = mybir.dt.bfloat16
    EXP = mybir.ActivationFunctionType.Exp
    LN = mybir.ActivationFunctionType.Ln

    nc = bacc.Bacc("TRN2", target_bir_lowering=False, debug=False)

    xT_d = nc.dram_tensor("xT", [128, KSUB, N], bf16, kind="ExternalInput")
    cT_d = nc.dram_tensor("ctxT", [128, KSUB, N], bf16, kind="ExternalInput")
    wq_d = nc.dram_tensor("wq", [128, KSUB, 2 * 128], bf16, kind="ExternalInput")
    wk_d = nc.dram_tensor("wk", [128, KSUB, 2 * 128], bf16, kind="ExternalInput")
    wv_d = nc.dram_tensor("wv", [128, KSUB, 2 * 128], bf16, kind="ExternalInput")
    wo_d = nc.dram_tensor("wo", [128, 2, DIM], bf16, kind="ExternalInput")
    cos_d = nc.dram_tensor("cosT", [128, N], bf16, kind="ExternalInput")
    sin_d = nc.dram_tensor("sinT", [128, N], bf16, kind="ExternalInput")
    y_d = nc.dram_tensor("y", [NT, 128, DIM], fp32, kind="ExternalOutput")

    with tile.TileContext(nc) as tc:
        with (
            tc.tile_pool(name="consts", bufs=1) as consts,
            tc.tile_pool(name="ps", bufs=2, space="PSUM") as ps,       # S tiles
            tc.tile_pool(name="pop", bufs=3, space="PSUM") as pop,     # AV accum
            tc.tile_pool(name="pjy", bufs=1, space="PSUM") as pjy,     # proj/v/y
            tc.tile_pool(name="ftmp", bufs=2) as ftmp,
            tc.tile_pool(name="upool", bufs=14) as upool,
            tc.tile_pool(name="stage", bufs=2) as stage,
            tc.tile_pool(name="ytail", bufs=2) as ytail,
            tc.tile_pool(name="rpool", bufs=4) as rpool,
        ):
          for _rep in range(reps):
            # ---- SBUF homes
            wk = consts.tile([128, KSUB, 256], bf16)
            wq = consts.tile([128, KSUB, 256], bf16)
            wv = consts.tile([128, KSUB, 256], bf16)
            wo = consts.tile([128, 2, DIM], bf16)
            cosT = consts.tile([128, N], bf16)
            sinT = consts.tile([128, N], bf16)
            xT = consts.tile([128, KSUB, N], bf16)
            ctxT = consts.tile([128, KSUB, N], bf16)

            H = N // 2
            # ---- input DMAs, ordered/chunked for earliest attention start
            # one DMA per (tensor, token-half): Sync-engine dispatch is
            # ~0.7us per dma_start, so few big strided transfers beat many
            # per-ksub chunks.
            nc.sync.dma_start(wk[:], wk_d[:])
            nc.sync.dma_start(wq[:], wq_d[:])
            nc.sync.dma_start(ctxT[:, :, 0:H], cT_d[:, :, 0:H])
            nc.sync.dma_start(cosT[:, 0:H], cos_d[:, 0:H])
            nc.sync.dma_start(sinT[:, 0:H], sin_d[:, 0:H])
            nc.sync.dma_start(xT[:, :, 0:H], xT_d[:, :, 0:H])
            nc.sync.dma_start(wv[:], wv_d[:])
            nc.sync.dma_start(ctxT[:, :, H:N], cT_d[:, :, H:N])
            nc.sync.dma_start(cosT[:, H:N], cos_d[:, H:N])
            nc.sync.dma_start(sinT[:, H:N], sin_d[:, H:N])
            nc.sync.dma_start(xT[:, :, H:N], xT_d[:, :, H:N])
            nc.sync.dma_start(wo[:], wo_d[:])

            # [part, head, ctx-tile, 64 v-dims + ones column]
            v_sb = consts.tile([128, G, NT, DH + 1], bf16)
            nc.gpsimd.memset(v_sb[:], 1.0)

            qrot = consts.tile([128, 2, N], bf16)   # [p, head-pair, n]
            krot = consts.tile([128, 2, N], bf16)
            ocat = consts.tile([128, 2, N], bf16)

            # ---- v projection: two token tiles per accumulator slot.
            # PSUM accumulation groups are bank-granular, so the two tiles'
            # groups (cols 0:256 / 256:512 of one 2KB bank) must run
            # SEQUENTIALLY: all 8 ks of tile jt, then all 8 of jt+1.
            def v_units(jt):
                box = {}

                def mm2(idx):
                    if idx == 0:
                        box["pv"] = pjy.tile([128, 512], fp32, tag="pjy",
                                             name="pv")
                    pv = box["pv"]
                    for k in (2 * idx, 2 * idx + 1):
                        jj, ks = divmod(k, KSUB)
                        t = jt + jj
                        nc.tensor.matmul(
                            pv[:, jj * 256:jj * 256 + 256],
                            ctxT[:, ks, t * 128:(t + 1) * 128], wv[:, ks, :],
                            start=(ks == 0), stop=(ks == KSUB - 1),
                        )

                def out():
                    nc.vector.tensor_copy(
                        v_sb[:, :, jt:jt + 2, 0:DH],
                        box["pv"][:].rearrange("p (j h d) -> p h j d",
                                               j=2, h=G),
                    )

                return [functools.partial(mm2, i) for i in range(KSUB)] + [out]

            # ---- q/k projection units (fd=512 chunks) + rotary
            def proj_units(w_sb, src, rot, hp, c):
                box = {}
                nsl = slice(c * 512, (c + 1) * 512)

                def mm2(ks):
                    if ks == 0:
                        box["pj"] = pjy.tile([128, 512], fp32, tag="pjy",
                                             name="pj")
                    for k in (ks, ks + 1):
                        nc.tensor.matmul(
                            box["pj"][:],
                            w_sb[:, k, hp * 128:(hp + 1) * 128],
                            src[:, k, nsl],
                            start=(k == 0), stop=(k == KSUB - 1),
                        )

                def rotary():
                    pj = box["pj"]
                    t1 = ftmp.tile([128, 512], fp32, tag="t1", name="t1")
                    t2 = ftmp.tile([128, 512], fp32, tag="t2", name="t2")
                    # shuffle first so pj's last reader comes early
                    nc.vector.stream_shuffle(t2[:], pj[:], SWAP_MASK)
                    nc.vector.tensor_mul(t1[:], pj[:], cosT[:, nsl])
                    nc.vector.tensor_mul(t2[:], t2[:], sinT[:, nsl])
                    nc.vector.tensor_add(rot[:, hp, nsl], t1[:], t2[:])

                return [functools.partial(mm2, ks)
                        for ks in range(0, KSUB, 2)] + [rotary]

            # ---- y projection units (fd=512), bounced through SBUF
            # (DMA cannot read PSUM)
            def y_units(t, c5):
                box = {}

                def mm():
                    box["py"] = pjy.tile([128, 512], fp32, tag="pjy", name="py")
                    for hp in range(2):
                        nc.tensor.matmul(
                            box["py"][:],
                            ocat[:, hp, t * 128:(t + 1) * 128],
                            wo[:, hp, c5 * 512:(c5 + 1) * 512],
                            start=(hp == 0), stop=(hp == 1),
                        )

                def out():
                    ysb = ytail.tile([128, 512], fp32, tag="ysh", name="ysh")
                    nc.vector.tensor_copy(ysb[:], box["py"][:])
                    nc.sync.dma_start(y_d[t][:, c5 * 512:(c5 + 1) * 512],
                                      ysb[:])

                return [mm, out]

            # tail y tiles reuse the (idle) sps slots at fd=1024, bounced
            # through SBUF so the slot frees at copy time.
            def y_tail(t):
                py = ps.tile([128, 1024], fp32, tag="sps", name="pyt")
                for c5 in range(2):
                    for hp in range(2):
                        nc.tensor.matmul(
                            py[:, c5 * 512:(c5 + 1) * 512],
                            ocat[:, hp, t * 128:(t + 1) * 128],
                            wo[:, hp, c5 * 512:(c5 + 1) * 512],
                            start=(hp == 0), stop=(hp == 1),
                        )
                ysb = ytail.tile([128, 1024], fp32, tag="ysb", name="ysb")
                # ACT is idle at the tail: split the copy across both engines
                nc.vector.tensor_copy(ysb[:, 0:512], py[:, 0:512])
                nc.scalar.copy(ysb[:, 512:1024], py[:, 512:1024])
                nc.sync.dma_start(y_d[t], ysb[:])

            # ---- the global stream
            filler = collections.deque()
            avq = collections.deque()   # (due_step, closure)

            def drain_filler(k):
                for _ in range(k):
                    if filler:
                        filler.popleft()()

            def drain_av(g):
                while avq and avq[0][0] <= g:
                    avq.popleft()[1]()

            def make_av(hp, c4, j, u, pobox):
                def run():
                    if j == 0:
                        pobox[0] = pop.tile([DH + 1, 512], fp32, tag="po",
                                            name="po0")
                        pobox[1] = pop.tile([DH + 1, 512], fp32, tag="po",
                                            name="po1")
                    for hh in range(2):
                        nc.tensor.matmul(
                            pobox[hh][:],
                            v_sb[:, 2 * hp + hh, j, :],
                            u[:, hh * 512:(hh + 1) * 512],
                            start=(j == 0), stop=(j == NT - 1),
                        )
                return run

            def make_normalize(hp, c4, pobox):
                qsl = slice(c4 * 512, (c4 + 1) * 512)

                def run():
                    # hh0's po slot is the one the next-next pass's AV needs
                    # first: stage it to SBUF immediately, normalize from there.
                    posb = stage.tile([DH + 1, 512], fp32, tag="posb",
                                      name="posb")
                    nc.vector.tensor_copy(posb[:], pobox[0][:])
                    srcs = (posb, pobox[1])
                    # the recip chain has ~a pass of slack (ocat consumed >=2
                    # passes later); demote it so filler-freeing DVE copies
                    # aren't stuck behind 3.3us reciprocals.
                    with tc.high_priority(offset=-100):
                        for hh in range(2):
                            r = hh * 64
                            src = srcs[hh]
                            rec = rpool.tile([1, 512], fp32, tag="rec",
                                             name="rec")
                            nc.vector.reciprocal(rec[:], src[DH:DH + 1, :])
                            rec64 = rpool.tile([DH, 512], fp32, tag="rec64",
                                               name="rec64")
                            nc.gpsimd.partition_broadcast(rec64[:], rec[:])
                            nc.vector.tensor_tensor(
                                ocat[r:r + 64, hp, qsl], src[0:DH, :],
                                rec64[:], mybir.AluOpType.mult)
                return run

            # ================= schedule =================
            # prefix: minimum work before the first attention pass
            for u_ in proj_units(wk, ctxT, krot, 0, 0):
                u_()
            for u_ in proj_units(wk, ctxT, krot, 0, 1):
                u_()
            for u_ in proj_units(wq, xT, qrot, 0, 0):
                u_()
            for u_ in v_units(0):        # tiles 0,1
                u_()

            passes = [(0, 0), (0, 1), (1, 0), (1, 1),
                      (0, 2), (1, 2), (0, 3), (1, 3)]
            # AV emission lag per pass. Decay is exactly 1/pass: pass p+1's
            # second po slot is pass p's first (bufs=3, 2 allocs/pass), so
            # AV(p+1)_0 must be emitted after AV(p)_15 + stage-copy (FIFO at
            # equal due-step keeps the order).
            lags = [12, 11, 10, 9, 8, 7, 6, 5]

            pass_fillers = [
                # P0 (hp0,q0)
                (v_units(2)
                 + proj_units(wk, ctxT, krot, 0, 2)
                 + proj_units(wk, ctxT, krot, 0, 3)
                 + v_units(4) + v_units(6)
                 + proj_units(wq, xT, qrot, 0, 1)
                 + v_units(8)),
                # P1 (hp0,q1)
                (v_units(10) + v_units(12) + v_units(14)
                 + proj_units(wk, ctxT, krot, 1, 0)
                 + proj_units(wq, xT, qrot, 1, 0)
                 + proj_units(wk, ctxT, krot, 1, 1)),
                # P2 (hp1,q0)
                (proj_units(wk, ctxT, krot, 1, 2)
                 + proj_units(wk, ctxT, krot, 1, 3)
                 + proj_units(wq, xT, qrot, 1, 1)),
                # P3 (hp1,q1)
                (proj_units(wq, xT, qrot, 0, 2)
                 + proj_units(wq, xT, qrot, 1, 2)
                 + proj_units(wq, xT, qrot, 0, 3)
                 + proj_units(wq, xT, qrot, 1, 3)),
                # P4 (hp0,q2): y for tokens 0..511 (final after P2)
                [u_ for t in range(0, 4) for c5 in range(2)
                 for u_ in y_units(t, c5)],
                # P5 (hp1,q2): y tokens 512..1023 (final after P3)
                [u_ for t in range(4, 8) for c5 in range(2)
                 for u_ in y_units(t, c5)],
                # P6 (hp0,q3): nothing — normalize(P5) only lands at g102
                # (AV lag), so y(8..11) must wait for P7
                [],
                # P7 (hp1,q3): y tokens 1024..1535 (final after P5's lagged
                # normalize)
                [u_ for t in range(8, 12) for c5 in range(2)
                 for u_ in y_units(t, c5)],
            ]

            g = 0
            for pi, (hp, c4) in enumerate(passes):
                filler.extend(pass_fillers[pi])
                qsl = slice(c4 * 512, (c4 + 1) * 512)
                pobox = {}
                budget = 3 if pi <= 2 else 2
                for j in range(NT):
                    drain_av(g)
                    sps = ps.tile([128, 1024], fp32, tag="sps", name="sps")
                    for hh in range(2):
                        r = hh * 64
                        nc.tensor.matmul(
                            sps[:, hh * 512:(hh + 1) * 512],
                            krot[r:r + 64, hp, j * 128:(j + 1) * 128],
                            qrot[r:r + 64, hp, qsl],
                            start=True, stop=True, tile_position=(r, 0),
                        )
                    u = upool.tile([128, 1024], bf16, tag="u", name="u")
                    nc.scalar.activation(u[:], sps[:], EXP, scale=SCALE)
                    avq.append((g + lags[pi], make_av(hp, c4, j, u, pobox)))
                    if j == NT - 1:
                        avq.append((g + lags[pi],
                                    make_normalize(hp, c4, pobox)))
                    drain_filler(budget)
                    g += 1

            # flush: remaining AV/normalize + fillers, then tail y
            while avq:
                drain_av(avq[0][0])
                drain_filler(2)
            while filler:
                filler.popleft()()
            for t in range(12, 16):
                y_tail(t)

    nc.finalize()
    return nc


def _prep_inputs(x, context, rotary_pos, Wq, Wkv, Wo):
    """Build the 8 per-core input maps (host-side shard + transpose + cast)."""
    x = np.asarray(x, dtype=np.float32)
    context = np.asarray(context, dtype=np.float32)
    rotary_pos = np.asarray(rotary_pos, dtype=np.float32)
    Wq = np.asarray(Wq, dtype=np.float32)
    Wkv = np.asarray(Wkv, dtype=np.float32)
    Wo = np.asarray(Wo, dtype=np.float32)

    Wk, Wv = Wkv[:, :DIM], Wkv[:, DIM:]

    cos = np.cos(rotary_pos).T.astype(np.float32)                # [64, n]
    sign = np.tile(np.array([-1.0, 1.0], np.float32), DH // 2)   # rotate_half sign
    sin = (np.sin(rotary_pos) * sign[None, :]).T.astype(np.float32)
    cosT = np.ascontiguousarray(np.concatenate([cos, cos], axis=0)).astype(BF16)
    sinT = np.ascontiguousarray(np.concatenate([sin, sin], axis=0)).astype(BF16)

    def to_kxm(w):  # [1024, 256] -> [128, 8, 256] (partition, ksub, m)
        return np.ascontiguousarray(
            w.reshape(KSUB, 128, w.shape[1]).transpose(1, 0, 2).astype(BF16))

    def to_pT(a):   # [2048, 1024] -> [128, 8, 2048]
        return np.ascontiguousarray(
            a.T.reshape(KSUB, 128, N).transpose(1, 0, 2).astype(BF16))

    in_maps = []
    for core in range(N_CORES):
        b, gg = divmod(core, G)
        cs = slice(gg * G * DH, (gg + 1) * G * DH)  # 256 cols of this head group
        in_maps.append({
            "xT": to_pT(x[b]),
            "ctxT": to_pT(context[b]),
            "wq": to_kxm(Wq[:, cs]),
            "wk": to_kxm(Wk[:, cs]),
            "wv": to_kxm(Wv[:, cs]),
            "wo": np.ascontiguousarray(
                Wo[cs, :].reshape(2, 128, DIM).transpose(1, 0, 2).astype(BF16)),
            "cosT": cosT,
            "sinT": sinT,
        })
    return in_maps


def _ensure_runner(reps=1):
    """Build the Bass program and a reusable jitted SPMD executor."""
    key = ("runner", reps)
    if key in _cached:
        return _cached[key]

    nc = _build_program(reps=reps)
    install_neuronx_cc_hook()
    partition_name = nc.partition_id_tensor.name if nc.partition_id_tensor else None

    in_names, out_names, out_avals = [], [], []
    for alloc in nc.m.functions[0].allocations:
        if not isinstance(alloc, mybir.MemoryLocationSet):
            continue
        name = alloc.memorylocations[0].name
        if alloc.kind == "ExternalInput":
            if name != partition_name:
                in_names.append(name)
        elif alloc.kind == "ExternalOutput":
            out_names.append(name)
            out_avals.append(jax.core.ShapedArray(
                tuple(alloc.tensor_shape), mybir.dt.np(alloc.dtype)))
    n_params = len(in_names)
    all_in_names = list(in_names) + list(out_names)
    if partition_name is not None:
        all_in_names.append(partition_name)

    def _body(*args):
        operands = list(args)
        if partition_name is not None:
            operands.append(bass2jax.partition_id_tensor())
        return tuple(_bass_exec_p.bind(
            *operands,
            out_avals=tuple(out_avals),
            in_names=tuple(all_in_names),
            out_names=tuple(out_names),
            lowering_input_output_aliases=(),
            sim_require_finite=True,
            sim_require_nnan=True,
            nc=nc,
        ))

    devices = jax.devices()[:N_CORES]
    mesh = Mesh(np.asarray(devices), ("core",))
    n_outs = len(out_names)
    donate = tuple(range(n_params, n_params + n_outs))
    sharded = jax.jit(
        shard_map(_body, mesh=mesh,
                  in_specs=(PartitionSpec("core"),) * (n_params + n_outs),
                  out_specs=(PartitionSpec("core"),) * n_outs,
                  check_rep=False),
        donate_argnums=donate,
        keep_unused=True,
    )

    import jax.numpy as jnp
    from jax.sharding import NamedSharding

    zero_shardings = tuple(
        NamedSharding(mesh, PartitionSpec("core")) for _ in out_avals)

    @functools.partial(jax.jit, out_shardings=zero_shardings)
    def zmaker():
        return tuple(
            jnp.zeros((N_CORES * a.shape[0], *a.shape[1:]), a.dtype)
            for a in out_avals)

    def exec_fn(concat_in):
        zeros = zmaker()
        outs = sharded(*concat_in, *zeros)
        jax.block_until_ready(outs)
        return outs

    _cached[key] = (exec_fn, in_names, out_names, out_avals,
                    sharded, zmaker)
    return _cached[key]


def _concat_inputs(in_maps, in_names):
    return [
        np.concatenate([np.asarray(in_maps[c][name]) for c in range(N_CORES)],
                       axis=0)
        for name in in_names
    ]


def _run(inputs, trace=False):
    exec_fn, in_names, out_names, out_avals = _ensure_runner()[:4]
    in_maps = _prep_inputs(
        inputs["x"], inputs["context"], inputs["rotary_pos"],
        inputs["Wq"], inputs["Wkv"], inputs["Wo"])
    outs = exec_fn(_concat_inputs(in_maps, in_names))

    yi = out_names.index("y")
    y_all = np.asarray(outs[yi]).reshape(N_CORES, *out_avals[yi].shape)

    bo = np.asarray(inputs["bo"], dtype=np.float32)
    y = np.zeros((B, N, DIM), dtype=np.float32)
    for core in range(N_CORES):
        y[core // G] += y_all[core].reshape(N, DIM)
    y += bo[None, None, :]
    return y, None


def kernel(**inputs) -> np.ndarray:
    y, _ = _run(inputs, trace=False)
    return y
